# revision 1
# baseline (speedup 1.0000x reference)
"""bf16 variant: 4-way PE-quadrant packing + once-per-image edge compute.

Main conv (d=4): four concurrent K=64/M=64 bf16 matmuls occupy the four 64x64
quadrants of the PE array (tile positions (0,0), (0,64), (64,0), (64,64)):
lower/upper image half on array rows, even/odd row-pair on array columns.
Each group of 4 row-pairs runs 9 taps x 4 units; evacuation (bias add,
psum->sbuf) alternates between ScalarE and VectorE.

Border pixels (the 1-wide frame uses convs d in {7,1,5,3,8,6,2,0}) are
computed once per image from a small host-packed edge tensor and written to a
separate output; the host overlays them onto the dense conv4 result.
"""

import ml_dtypes
import numpy as np

import concourse.bacc as bacc
import concourse.mybir as mybir
import concourse.tile as tile
from concourse.bass import ts
from concourse.bass_utils import run_bass_kernel_spmd

B, C, H, W = 8, 64, 256, 256
NCORES = 8
R = 32              # output rows per strip
H2 = R // 2         # rows per partition-half
NSTRIP = H // R
SLOTS = H2 + 2
WPAD = W + 2
XCOLS = SLOTS * WPAD
NG = H2 // 4        # groups of 4 row-pairs per half
F32 = mybir.dt.float32
BF16 = mybir.dt.bfloat16
AF = mybir.ActivationFunctionType
BF = ml_dtypes.bfloat16

TAPS9 = [(dy, dx) for dy in (-1, 0, 1) for dx in (-1, 0, 1)]
TOP_TAPS = [(dy, dx) for dy in (0, 1) for dx in (-1, 0, 1)]      # d=7 row 0
BOT_TAPS = [(dy, dx) for dy in (-1, 0) for dx in (-1, 0, 1)]     # d=1 row 255
LEFT_TAPS = [(dy, dx) for dy in (-1, 0, 1) for dx in (0, 1)]     # d=5 col 0
RIGHT_TAPS = [(dy, dx) for dy in (-1, 0, 1) for dx in (-1, 0)]   # d=3 col 255
C6_TAPS = [(dy, dx) for dy in (0, 1) for dx in (-1, 0)]          # d=6 (0,255)
C2_TAPS = [(dy, dx) for dy in (-1, 0) for dx in (0, 1)]          # d=2 (255,0)
C8_TAPS = [(dy, dx) for dy in (0, 1) for dx in (0, 1)]           # d=8 (0,0)
C0_TAPS = [(dy, dx) for dy in (-1, 0) for dx in (-1, 0)]         # d=0 (255,255)

W_GROUPS = [
    (4, TAPS9), (7, TOP_TAPS), (1, BOT_TAPS), (5, LEFT_TAPS),
    (3, RIGHT_TAPS), (6, C6_TAPS), (2, C2_TAPS), (8, C8_TAPS), (0, C0_TAPS),
]
_offs = []
_acc = 0
for _d, _taps in W_GROUPS:
    _offs.append(_acc)
    _acc += len(_taps)
(MAIN_S, TOP_S, BOT_S, LEFT_S, RIGHT_S, C6_S, C2_S, C8_S, C0_S) = _offs
NW = _acc  # 49
# bias tile [128, NB]: column -> (value on partitions 0-63, on 64-127)
BIAS_PAIRS = [(4, 4), (5, 5), (3, 3), (7, 1), (8, 2), (6, 0)]
B_MAIN, B_LEFT, B_RIGHT, B_TOPBOT, B_C82, B_C60 = range(6)
NB = 6

# edge_in sections (element offsets per partition)
LCOL, RCOL, TOPS, BOTS = 0, 516, 1032, 1548
EIN = 2064
EOUT = 516  # edges_out: [0:128] left, [128:256] right, [256:512] top|bottom,
            # [512:514] corner j01 (d8|d2), [514:516] corner j254/255 (d6|d0)

_CACHE = {}


def _chain(nc, psd, wtr, wslice, slot0, taps, rhs_fn):
    n = len(taps)
    for k, (dy, dx) in enumerate(taps):
        nc.tensor.matmul(psd, wtr[wslice, ts(slot0 + k, 64)], rhs_fn(dy, dx),
                         start=(k == 0), stop=(k == n - 1),
                         skip_group_check=True)


def _build():
    nc = bacc.Bacc("TRN2", target_bir_lowering=False, debug=False,
                   num_devices=NCORES)
    ip = nc.dram_tensor("img_prep", [NSTRIP, 128, XCOLS], BF16,
                        kind="ExternalInput").ap()
    ein_d = nc.dram_tensor("edge_in", [128, EIN], BF16,
                           kind="ExternalInput").ap()
    wt_d = nc.dram_tensor("wt", [128, NW * 64], BF16, kind="ExternalInput").ap()
    bias_d = nc.dram_tensor("bias", [128, NB], F32, kind="ExternalInput").ap()
    out_d = nc.dram_tensor("out", [NSTRIP, 2, 128, H2 * W // 2], F32,
                           kind="ExternalOutput").ap()
    edg_d = nc.dram_tensor("edges", [128, EOUT], F32,
                           kind="ExternalOutput").ap()

    lo, up = slice(0, 64), slice(64, 128)

    with tile.TileContext(nc) as tc:
        with (
            tc.tile_pool(name="const", bufs=1) as constp,
            tc.tile_pool(name="xin", bufs=5) as xp,
            tc.tile_pool(name="outp", bufs=5) as op,
            tc.tile_pool(name="psmain", bufs=4, space="PSUM") as pp,
        ):
            wt = constp.tile([128, NW * 64], BF16)
            nc.sync.dma_start(wt[:], wt_d[:])
            bias_t = constp.tile([128, NB], F32)
            nc.sync.dma_start(bias_t[:], bias_d[:])
            ein = constp.tile([128, EIN], BF16)
            nc.sync.dma_start(ein[:], ein_d[:])
            wtr = wt[:]

            # ---- edge computation (once per image) ----
            Elc = ein[:, LCOL:LCOL + 516].rearrange("p (t k) -> p t k", k=2)
            Erc = ein[:, RCOL:RCOL + 516].rearrange("p (t k) -> p t k", k=2)
            Et = ein[:, TOPS:TOPS + 516].rearrange("p (r m) -> p r m", m=WPAD)
            Eb = ein[:, BOTS:BOTS + 516].rearrange("p (r m) -> p r m", m=WPAD)
            esb = constp.tile([128, EOUT], F32)

            # left column (d=5): rows 0-127 on (0,0), rows 128-255 on (64,64)
            pside = pp.tile([128, 256], F32, tag="ps1")
            _chain(nc, pside[lo, 0:128], wtr, lo, LEFT_S, LEFT_TAPS,
                   lambda dy, dx: Elc[lo, 1 + dy: 129 + dy, dx])
            _chain(nc, pside[up, 0:128], wtr, up, LEFT_S, LEFT_TAPS,
                   lambda dy, dx: Elc[up, 129 + dy: 257 + dy, dx])
            nc.scalar.activation(esb[:, 0:128], pside[:, 0:128], AF.Identity,
                                 bias=bias_t[:, B_LEFT:B_LEFT + 1])
            # right column (d=3): img col 255+dx is k index dx+1
            pside2 = pp.tile([128, 256], F32, tag="ps2")
            _chain(nc, pside2[lo, 0:128], wtr, lo, RIGHT_S, RIGHT_TAPS,
                   lambda dy, dx: Erc[lo, 1 + dy: 129 + dy, dx + 1])
            _chain(nc, pside2[up, 0:128], wtr, up, RIGHT_S, RIGHT_TAPS,
                   lambda dy, dx: Erc[up, 129 + dy: 257 + dy, dx + 1])
            nc.scalar.activation(esb[:, 128:256], pside2[:, 0:128], AF.Identity,
                                 bias=bias_t[:, B_RIGHT:B_RIGHT + 1])
            # top row (d=7) on (0,0) / bottom row (d=1) on (64,64)
            ptb = pp.tile([128, 256], F32, tag="ps1")
            _chain(nc, ptb[lo, 0:256], wtr, lo, TOP_S, TOP_TAPS,
                   lambda dy, dx: Et[lo, dy, dx + 1: dx + 257])
            _chain(nc, ptb[up, 0:256], wtr, up, BOT_S, BOT_TAPS,
                   lambda dy, dx: Eb[up, 1 + dy, dx + 1: dx + 257])
            nc.scalar.activation(esb[:, 256:512], ptb[:, 0:256], AF.Identity,
                                 bias=bias_t[:, B_TOPBOT:B_TOPBOT + 1])
            # corners: (0,0) d8 / (255,0) d2 at cols 512:514;
            #          (0,255) d6 / (255,255) d0 at cols 514:516
            pcn = pp.tile([128, 256], F32, tag="ps2")
            _chain(nc, pcn[lo, 0:2], wtr, lo, C8_S, C8_TAPS,
                   lambda dy, dx: Et[lo, dy, dx + 1: dx + 3])
            _chain(nc, pcn[up, 0:2], wtr, up, C2_S, C2_TAPS,
                   lambda dy, dx: Eb[up, 1 + dy, dx + 1: dx + 3])
            _chain(nc, pcn[lo, 2:4], wtr, lo, C6_S, C6_TAPS,
                   lambda dy, dx: Et[lo, dy, 255 + dx: 257 + dx])
            _chain(nc, pcn[up, 2:4], wtr, up, C0_S, C0_TAPS,
                   lambda dy, dx: Eb[up, 1 + dy, 255 + dx: 257 + dx])
            nc.scalar.activation(esb[:, 512:514], pcn[:, 0:2], AF.Identity,
                                 bias=bias_t[:, B_C82:B_C82 + 1])
            nc.scalar.activation(esb[:, 514:516], pcn[:, 2:4], AF.Identity,
                                 bias=bias_t[:, B_C60:B_C60 + 1])
            nc.sync.dma_start(edg_d[:], esb[:])

            # ---- dense interior conv (d=4) ----
            for s in range(NSTRIP):
                X = xp.tile([128, XCOLS], BF16)
                nc.sync.dma_start(X[:], ip[s])
                Xv = X[:].rearrange("p (t m) -> p t m", m=WPAD)
                olo = op.tile([128, H2 * W // 2], F32, tag="olo")
                oup = op.tile([128, H2 * W // 2], F32, tag="oup")

                for g in range(NG):
                    ps1 = pp.tile([128, 512], F32, tag="ps1")
                    ps2 = pp.tile([128, 512], F32, tag="ps2")
                    for k, (dy, dx) in enumerate(TAPS9):
                        st, sp = (k == 0), (k == 8)
                        for (ph, po, i) in ((lo, slice(0, 64), 4 * g),
                                            (up, slice(0, 64), 4 * g),
                                            (lo, slice(64, 128), 4 * g + 2),
                                            (up, slice(64, 128), 4 * g + 2)):
                            psd = (ps1 if ph == lo else ps2)
                            rhs = Xv[ph, i + 1 + dy: i + 3 + dy,
                                     dx + 1: dx + 257]
                            nc.tensor.matmul(
                                psd[po, :],
                                wtr[ph, ts(MAIN_S + k, 64)], rhs,
                                start=st, stop=sp, skip_group_check=True)
                    # evacuate: bias add psum -> sbuf; alternate engines
                    blo = bias_t[:, B_MAIN:B_MAIN + 1]
                    if g % 2 == 0:
                        nc.scalar.activation(olo[:, ts(g, 512)], ps1[:],
                                             AF.Identity, bias=blo)
                        nc.vector.tensor_scalar_add(oup[:, ts(g, 512)],
                                                    ps2[:], blo)
                    else:
                        nc.vector.tensor_scalar_add(olo[:, ts(g, 512)],
                                                    ps1[:], blo)
                        nc.scalar.activation(oup[:, ts(g, 512)], ps2[:],
                                             AF.Identity, bias=blo)

                nc.sync.dma_start(out_d[s, 0], olo[:])
                nc.sync.dma_start(out_d[s, 1], oup[:])



    nc.compile()
    return nc


def _get_nc():
    if "nc" not in _CACHE:
        _CACHE["nc"] = _build()
    return _CACHE["nc"]


def _prep_img(imgc):
    """[64,256,256] f32 -> [NSTRIP,128,XCOLS] padded bf16 strip layout."""
    ip = np.zeros((NSTRIP, 2, 64, SLOTS, WPAD), BF)
    for s in range(NSTRIP):
        for g in range(2):
            base = s * R + g * H2 - 1
            l0 = max(0, -base)
            h0 = min(SLOTS, H - base)
            ip[s, g, :, l0:h0, 1:257] = imgc[:, base + l0: base + h0, :]
    return np.ascontiguousarray(ip.reshape(NSTRIP, 128, XCOLS))


def _prep_edge_in(imgc):
    L = np.zeros((64, 258, 2), BF)
    L[:, 1:257, :] = imgc[:, :, 0:2]
    Rt = np.zeros((64, 258, 2), BF)
    Rt[:, 1:257, :] = imgc[:, :, 254:256]
    T = np.zeros((64, 2, WPAD), BF)
    T[:, :, 1:257] = imgc[:, 0:2, :]
    Bo = np.zeros((64, 2, WPAD), BF)
    Bo[:, :, 1:257] = imgc[:, 254:256, :]
    half = np.concatenate([L.reshape(64, 516), Rt.reshape(64, 516),
                           T.reshape(64, 516), Bo.reshape(64, 516)], axis=1)
    return np.ascontiguousarray(np.concatenate([half, half], axis=0))


def _prep_wt(weights):
    wt = np.zeros((128, NW, 64), BF)
    for (d, taps), base in zip(W_GROUPS, _offs):
        for k, (dy, dx) in enumerate(taps):
            m = weights[d][:, :, dy + 1, dx + 1].T  # [cin, cout]
            wt[0:64, base + k] = m
            wt[64:128, base + k] = m
    return np.ascontiguousarray(wt.reshape(128, NW * 64))


def _prep_bias(bias):
    bs = np.zeros((128, NB), np.float32)
    for c, (dl, du) in enumerate(BIAS_PAIRS):
        bs[0:64, c] = bias[dl]
        bs[64:128, c] = bias[du]
    return bs


def _make_in_maps(img, weights, bias):
    img = np.asarray(img, np.float32)
    wt = _prep_wt(np.asarray(weights, np.float32))
    bs = _prep_bias(np.asarray(bias, np.float32))
    return [{"img_prep": _prep_img(img[c]), "edge_in": _prep_edge_in(img[c]),
             "wt": wt, "bias": bs}
            for c in range(NCORES)]


def _unprep_out(o, e):
    """Assemble [C,H,W] from dense out + edge overlay."""
    v = o.reshape(NSTRIP, 2, 2, 64, 4, 2, 256)  # s half par c g r w
    out = np.ascontiguousarray(
        v.transpose(3, 0, 1, 4, 2, 5, 6).reshape(C, H, W))
    Lv = np.concatenate([e[0:64, 0:128], e[64:128, 0:128]], axis=1)
    Rv = np.concatenate([e[0:64, 128:256], e[64:128, 128:256]], axis=1)
    out[:, 1:255, 0] = Lv[:, 1:255]
    out[:, 1:255, 255] = Rv[:, 1:255]
    out[:, 0, 1:255] = e[0:64, 257:511]
    out[:, 255, 1:255] = e[64:128, 257:511]
    out[:, 0, 0] = e[0:64, 512]
    out[:, 255, 0] = e[64:128, 512]
    out[:, 0, 255] = e[0:64, 515]
    out[:, 255, 255] = e[64:128, 515]
    return out


def kernel(img, weights, bias):
    nc = _get_nc()
    in_maps = _make_in_maps(img, weights, bias)
    res = run_bass_kernel_spmd(nc, in_maps, list(range(NCORES)))
    return np.stack([_unprep_out(res.results[c]["out"], res.results[c]["edges"])
                     for c in range(NCORES)])



# revision 4
# speedup vs baseline: 1.0248x; 1.0248x over previous
"""bf16 variant: 4-way PE-quadrant packing + once-per-image edge compute.

Main conv (d=4): four concurrent K=64/M=64 bf16 matmuls occupy the four 64x64
quadrants of the PE array (tile positions (0,0), (0,64), (64,0), (64,64)):
lower/upper image half on array rows, even/odd row-pair on array columns.
Each group of 4 row-pairs runs 9 taps x 4 units; evacuation (bias add,
psum->sbuf) alternates between ScalarE and VectorE.

Border pixels (the 1-wide frame uses convs d in {7,1,5,3,8,6,2,0}) are
computed once per image from a small host-packed edge tensor and written to a
separate output; the host overlays them onto the dense conv4 result.
"""

import ml_dtypes
import numpy as np

import concourse.bacc as bacc
import concourse.mybir as mybir
import concourse.tile as tile
from concourse.bass import ts
from concourse.bass_utils import run_bass_kernel_spmd

B, C, H, W = 8, 64, 256, 256
NCORES = 8
R = 32              # output rows per strip
H2 = R // 2         # rows per partition-half
NSTRIP = H // R
SLOTS = H2 + 2
WPAD = W + 2
XCOLS = SLOTS * WPAD
NG = H2 // 4        # groups of 4 row-pairs per half
F32 = mybir.dt.float32
BF16 = mybir.dt.bfloat16
AF = mybir.ActivationFunctionType
BF = ml_dtypes.bfloat16

TAPS9 = [(dy, dx) for dy in (-1, 0, 1) for dx in (-1, 0, 1)]
TOP_TAPS = [(dy, dx) for dy in (0, 1) for dx in (-1, 0, 1)]      # d=7 row 0
BOT_TAPS = [(dy, dx) for dy in (-1, 0) for dx in (-1, 0, 1)]     # d=1 row 255
LEFT_TAPS = [(dy, dx) for dy in (-1, 0, 1) for dx in (0, 1)]     # d=5 col 0
RIGHT_TAPS = [(dy, dx) for dy in (-1, 0, 1) for dx in (-1, 0)]   # d=3 col 255
C6_TAPS = [(dy, dx) for dy in (0, 1) for dx in (-1, 0)]          # d=6 (0,255)
C2_TAPS = [(dy, dx) for dy in (-1, 0) for dx in (0, 1)]          # d=2 (255,0)
C8_TAPS = [(dy, dx) for dy in (0, 1) for dx in (0, 1)]           # d=8 (0,0)
C0_TAPS = [(dy, dx) for dy in (-1, 0) for dx in (-1, 0)]         # d=0 (255,255)

W_GROUPS = [
    (4, TAPS9), (7, TOP_TAPS), (1, BOT_TAPS), (5, LEFT_TAPS),
    (3, RIGHT_TAPS), (6, C6_TAPS), (2, C2_TAPS), (8, C8_TAPS), (0, C0_TAPS),
]
_offs = []
_acc = 0
for _d, _taps in W_GROUPS:
    _offs.append(_acc)
    _acc += len(_taps)
(MAIN_S, TOP_S, BOT_S, LEFT_S, RIGHT_S, C6_S, C2_S, C8_S, C0_S) = _offs
NW = _acc  # 49
# bias tile [128, NB]: column -> (value on partitions 0-63, on 64-127)
BIAS_PAIRS = [(4, 4), (5, 5), (3, 3), (7, 1), (8, 2), (6, 0)]
B_MAIN, B_LEFT, B_RIGHT, B_TOPBOT, B_C82, B_C60 = range(6)
NB = 6

# edge_in sections (element offsets per partition)
LCOL, RCOL, TOPS, BOTS = 0, 516, 1032, 1548
EIN = 2064
EOUT = 516  # edges_out: [0:128] left, [128:256] right, [256:512] top|bottom,
            # [512:514] corner j01 (d8|d2), [514:516] corner j254/255 (d6|d0)

_CACHE = {}


def _chain(nc, psd, wtr, wslice, slot0, taps, rhs_fn):
    n = len(taps)
    for k, (dy, dx) in enumerate(taps):
        nc.tensor.matmul(psd, wtr[wslice, ts(slot0 + k, 64)], rhs_fn(dy, dx),
                         start=(k == 0), stop=(k == n - 1),
                         skip_group_check=True)


def _build():
    nc = bacc.Bacc("TRN2", target_bir_lowering=False, debug=False,
                   num_devices=NCORES)
    ip = nc.dram_tensor("img_prep", [NSTRIP, 128, XCOLS], BF16,
                        kind="ExternalInput").ap()
    ein_d = nc.dram_tensor("edge_in", [128, EIN], BF16,
                           kind="ExternalInput").ap()
    wt_d = nc.dram_tensor("wt", [128, NW * 64], BF16, kind="ExternalInput").ap()
    bias_d = nc.dram_tensor("bias", [128, NB], F32, kind="ExternalInput").ap()
    out_d = nc.dram_tensor("out", [NSTRIP, 2, 128, H2 * W // 2], BF16,
                           kind="ExternalOutput").ap()
    edg_d = nc.dram_tensor("edges", [128, EOUT], F32,
                           kind="ExternalOutput").ap()

    lo, up = slice(0, 64), slice(64, 128)

    with tile.TileContext(nc) as tc:
        with (
            tc.tile_pool(name="const", bufs=1) as constp,
            tc.tile_pool(name="xin", bufs=5) as xp,
            tc.tile_pool(name="outp", bufs=5) as op,
            tc.tile_pool(name="psmain", bufs=4, space="PSUM") as pp,
        ):
            wt = constp.tile([128, NW * 64], BF16)
            nc.sync.dma_start(wt[:], wt_d[:])
            bias_t = constp.tile([128, NB], F32)
            nc.sync.dma_start(bias_t[:], bias_d[:])
            ein = constp.tile([128, EIN], BF16)
            nc.sync.dma_start(ein[:], ein_d[:])
            wtr = wt[:]

            # ---- edge computation (once per image) ----
            Elc = ein[:, LCOL:LCOL + 516].rearrange("p (t k) -> p t k", k=2)
            Erc = ein[:, RCOL:RCOL + 516].rearrange("p (t k) -> p t k", k=2)
            Et = ein[:, TOPS:TOPS + 516].rearrange("p (r m) -> p r m", m=WPAD)
            Eb = ein[:, BOTS:BOTS + 516].rearrange("p (r m) -> p r m", m=WPAD)
            esb = constp.tile([128, EOUT], F32)

            # left column (d=5): rows 0-127 on (0,0), rows 128-255 on (64,64)
            pside = pp.tile([128, 256], F32, tag="ps1")
            _chain(nc, pside[lo, 0:128], wtr, lo, LEFT_S, LEFT_TAPS,
                   lambda dy, dx: Elc[lo, 1 + dy: 129 + dy, dx])
            _chain(nc, pside[up, 0:128], wtr, up, LEFT_S, LEFT_TAPS,
                   lambda dy, dx: Elc[up, 129 + dy: 257 + dy, dx])
            nc.scalar.activation(esb[:, 0:128], pside[:, 0:128], AF.Identity,
                                 bias=bias_t[:, B_LEFT:B_LEFT + 1])
            # right column (d=3): img col 255+dx is k index dx+1
            pside2 = pp.tile([128, 256], F32, tag="ps2")
            _chain(nc, pside2[lo, 0:128], wtr, lo, RIGHT_S, RIGHT_TAPS,
                   lambda dy, dx: Erc[lo, 1 + dy: 129 + dy, dx + 1])
            _chain(nc, pside2[up, 0:128], wtr, up, RIGHT_S, RIGHT_TAPS,
                   lambda dy, dx: Erc[up, 129 + dy: 257 + dy, dx + 1])
            nc.scalar.activation(esb[:, 128:256], pside2[:, 0:128], AF.Identity,
                                 bias=bias_t[:, B_RIGHT:B_RIGHT + 1])
            # top row (d=7) on (0,0) / bottom row (d=1) on (64,64)
            ptb = pp.tile([128, 256], F32, tag="ps1")
            _chain(nc, ptb[lo, 0:256], wtr, lo, TOP_S, TOP_TAPS,
                   lambda dy, dx: Et[lo, dy, dx + 1: dx + 257])
            _chain(nc, ptb[up, 0:256], wtr, up, BOT_S, BOT_TAPS,
                   lambda dy, dx: Eb[up, 1 + dy, dx + 1: dx + 257])
            nc.scalar.activation(esb[:, 256:512], ptb[:, 0:256], AF.Identity,
                                 bias=bias_t[:, B_TOPBOT:B_TOPBOT + 1])
            # corners: (0,0) d8 / (255,0) d2 at cols 512:514;
            #          (0,255) d6 / (255,255) d0 at cols 514:516
            pcn = pp.tile([128, 256], F32, tag="ps2")
            _chain(nc, pcn[lo, 0:2], wtr, lo, C8_S, C8_TAPS,
                   lambda dy, dx: Et[lo, dy, dx + 1: dx + 3])
            _chain(nc, pcn[up, 0:2], wtr, up, C2_S, C2_TAPS,
                   lambda dy, dx: Eb[up, 1 + dy, dx + 1: dx + 3])
            _chain(nc, pcn[lo, 2:4], wtr, lo, C6_S, C6_TAPS,
                   lambda dy, dx: Et[lo, dy, 255 + dx: 257 + dx])
            _chain(nc, pcn[up, 2:4], wtr, up, C0_S, C0_TAPS,
                   lambda dy, dx: Eb[up, 1 + dy, 255 + dx: 257 + dx])
            nc.scalar.activation(esb[:, 512:514], pcn[:, 0:2], AF.Identity,
                                 bias=bias_t[:, B_C82:B_C82 + 1])
            nc.scalar.activation(esb[:, 514:516], pcn[:, 2:4], AF.Identity,
                                 bias=bias_t[:, B_C60:B_C60 + 1])
            nc.sync.dma_start(edg_d[:], esb[:])

            # ---- dense interior conv (d=4) ----
            for s in range(NSTRIP):
                X = xp.tile([128, XCOLS], BF16)
                nc.sync.dma_start(X[:], ip[s])
                Xv = X[:].rearrange("p (t m) -> p t m", m=WPAD)
                olo = op.tile([128, H2 * W // 2], BF16, tag="olo")
                oup = op.tile([128, H2 * W // 2], BF16, tag="oup")

                for g in range(NG):
                    ps1 = pp.tile([128, 512], F32, tag="ps1")
                    ps2 = pp.tile([128, 512], F32, tag="ps2")
                    for k, (dy, dx) in enumerate(TAPS9):
                        st, sp = (k == 0), (k == 8)
                        for (ph, po, i) in ((lo, slice(0, 64), 4 * g),
                                            (up, slice(0, 64), 4 * g),
                                            (lo, slice(64, 128), 4 * g + 2),
                                            (up, slice(64, 128), 4 * g + 2)):
                            psd = (ps1 if ph == lo else ps2)
                            rhs = Xv[ph, i + 1 + dy: i + 3 + dy,
                                     dx + 1: dx + 257]
                            nc.tensor.matmul(
                                psd[po, :],
                                wtr[ph, ts(MAIN_S + k, 64)], rhs,
                                start=st, stop=sp, skip_group_check=True)
                    # evacuate: bias add psum -> sbuf; alternate engines
                    blo = bias_t[:, B_MAIN:B_MAIN + 1]
                    if g % 2 == 0:
                        nc.scalar.activation(olo[:, ts(g, 512)], ps1[:],
                                             AF.Identity, bias=blo)
                        nc.vector.tensor_scalar_add(oup[:, ts(g, 512)],
                                                    ps2[:], blo)
                    else:
                        nc.vector.tensor_scalar_add(olo[:, ts(g, 512)],
                                                    ps1[:], blo)
                        nc.scalar.activation(oup[:, ts(g, 512)], ps2[:],
                                             AF.Identity, bias=blo)

                nc.sync.dma_start(out_d[s, 0], olo[:])
                nc.sync.dma_start(out_d[s, 1], oup[:])



    nc.compile()
    return nc


def _get_nc():
    if "nc" not in _CACHE:
        _CACHE["nc"] = _build()
    return _CACHE["nc"]


def _prep_img(imgc):
    """[64,256,256] f32 -> [NSTRIP,128,XCOLS] padded bf16 strip layout."""
    ip = np.zeros((NSTRIP, 2, 64, SLOTS, WPAD), BF)
    for s in range(NSTRIP):
        for g in range(2):
            base = s * R + g * H2 - 1
            l0 = max(0, -base)
            h0 = min(SLOTS, H - base)
            ip[s, g, :, l0:h0, 1:257] = imgc[:, base + l0: base + h0, :]
    return np.ascontiguousarray(ip.reshape(NSTRIP, 128, XCOLS))


def _prep_edge_in(imgc):
    L = np.zeros((64, 258, 2), BF)
    L[:, 1:257, :] = imgc[:, :, 0:2]
    Rt = np.zeros((64, 258, 2), BF)
    Rt[:, 1:257, :] = imgc[:, :, 254:256]
    T = np.zeros((64, 2, WPAD), BF)
    T[:, :, 1:257] = imgc[:, 0:2, :]
    Bo = np.zeros((64, 2, WPAD), BF)
    Bo[:, :, 1:257] = imgc[:, 254:256, :]
    half = np.concatenate([L.reshape(64, 516), Rt.reshape(64, 516),
                           T.reshape(64, 516), Bo.reshape(64, 516)], axis=1)
    return np.ascontiguousarray(np.concatenate([half, half], axis=0))


def _prep_wt(weights):
    wt = np.zeros((128, NW, 64), BF)
    for (d, taps), base in zip(W_GROUPS, _offs):
        for k, (dy, dx) in enumerate(taps):
            m = weights[d][:, :, dy + 1, dx + 1].T  # [cin, cout]
            wt[0:64, base + k] = m
            wt[64:128, base + k] = m
    return np.ascontiguousarray(wt.reshape(128, NW * 64))


def _prep_bias(bias):
    bs = np.zeros((128, NB), np.float32)
    for c, (dl, du) in enumerate(BIAS_PAIRS):
        bs[0:64, c] = bias[dl]
        bs[64:128, c] = bias[du]
    return bs


def _make_in_maps(img, weights, bias):
    img = np.asarray(img, np.float32)
    wt = _prep_wt(np.asarray(weights, np.float32))
    bs = _prep_bias(np.asarray(bias, np.float32))
    return [{"img_prep": _prep_img(img[c]), "edge_in": _prep_edge_in(img[c]),
             "wt": wt, "bias": bs}
            for c in range(NCORES)]


def _unprep_out(o, e):
    """Assemble [C,H,W] from dense out + edge overlay."""
    v = o.astype(np.float32).reshape(NSTRIP, 2, 2, 64, 4, 2, 256)  # s half par c g r w
    out = np.ascontiguousarray(
        v.transpose(3, 0, 1, 4, 2, 5, 6).reshape(C, H, W))
    Lv = np.concatenate([e[0:64, 0:128], e[64:128, 0:128]], axis=1)
    Rv = np.concatenate([e[0:64, 128:256], e[64:128, 128:256]], axis=1)
    out[:, 1:255, 0] = Lv[:, 1:255]
    out[:, 1:255, 255] = Rv[:, 1:255]
    out[:, 0, 1:255] = e[0:64, 257:511]
    out[:, 255, 1:255] = e[64:128, 257:511]
    out[:, 0, 0] = e[0:64, 512]
    out[:, 255, 0] = e[64:128, 512]
    out[:, 0, 255] = e[0:64, 515]
    out[:, 255, 255] = e[64:128, 515]
    return out


def kernel(img, weights, bias):
    nc = _get_nc()
    in_maps = _make_in_maps(img, weights, bias)
    res = run_bass_kernel_spmd(nc, in_maps, list(range(NCORES)))
    return np.stack([_unprep_out(res.results[c]["out"], res.results[c]["edges"])
                     for c in range(NCORES)])



# revision 8
# speedup vs baseline: 1.0965x; 1.0699x over previous
"""bf16 variant: 4-way PE-quadrant packing + once-per-image edge compute.

Main conv (d=4): four concurrent K=64/M=64 bf16 matmuls occupy the four 64x64
quadrants of the PE array (tile positions (0,0), (0,64), (64,0), (64,64)):
lower/upper image half on array rows, even/odd row-pair on array columns.
Each group of 4 row-pairs runs 9 taps x 4 units; evacuation (bias add,
psum->sbuf) alternates between ScalarE and VectorE.

Startup critical path: main-tap weights (148KB) and the first strip (in 3
chunks) are DMA'd first; edge tensors stream in behind strips 1-2. Border
pixels (the 1-wide frame uses convs d in {7,1,5,3,8,6,2,0}) are computed
once per image AFTER the dense loop (overlapping the output-DMA drain) and
written to a separate output; the host overlays them onto the dense result.
Dense + edge outputs are bf16 (error budget allows it; halves out traffic).
"""

import ml_dtypes
import numpy as np

import concourse.bacc as bacc
import concourse.mybir as mybir
import concourse.tile as tile
from concourse.bass import ts
from concourse.bass_utils import run_bass_kernel_spmd

B, C, H, W = 8, 64, 256, 256
NCORES = 8
R = 32              # output rows per strip
H2 = R // 2         # rows per partition-half
NSTRIP = H // R
SLOTS = H2 + 2
WPAD = W + 2
XCOLS = SLOTS * WPAD
NG = H2 // 4        # groups of 4 row-pairs per half
F32 = mybir.dt.float32
BF16 = mybir.dt.bfloat16
AF = mybir.ActivationFunctionType
BF = ml_dtypes.bfloat16

TAPS9 = [(dy, dx) for dy in (-1, 0, 1) for dx in (-1, 0, 1)]
TOP_TAPS = [(dy, dx) for dy in (0, 1) for dx in (-1, 0, 1)]      # d=7 row 0
BOT_TAPS = [(dy, dx) for dy in (-1, 0) for dx in (-1, 0, 1)]     # d=1 row 255
LEFT_TAPS = [(dy, dx) for dy in (-1, 0, 1) for dx in (0, 1)]     # d=5 col 0
RIGHT_TAPS = [(dy, dx) for dy in (-1, 0, 1) for dx in (-1, 0)]   # d=3 col 255
C6_TAPS = [(dy, dx) for dy in (0, 1) for dx in (-1, 0)]          # d=6 (0,255)
C2_TAPS = [(dy, dx) for dy in (-1, 0) for dx in (0, 1)]          # d=2 (255,0)
C8_TAPS = [(dy, dx) for dy in (0, 1) for dx in (0, 1)]           # d=8 (0,0)
C0_TAPS = [(dy, dx) for dy in (-1, 0) for dx in (-1, 0)]         # d=0 (255,255)

W_GROUPS = [
    (4, TAPS9), (7, TOP_TAPS), (1, BOT_TAPS), (5, LEFT_TAPS),
    (3, RIGHT_TAPS), (6, C6_TAPS), (2, C2_TAPS), (8, C8_TAPS), (0, C0_TAPS),
]
_offs = []
_acc = 0
for _d, _taps in W_GROUPS:
    _offs.append(_acc)
    _acc += len(_taps)
(MAIN_S, TOP_S, BOT_S, LEFT_S, RIGHT_S, C6_S, C2_S, C8_S, C0_S) = _offs
NW = _acc  # 49
NWM = len(TAPS9)       # main-tap weight columns (first group)
NWE = NW - NWM         # edge-tap weight columns
# bias tile [128, NB]: column -> (value on partitions 0-63, on 64-127)
BIAS_PAIRS = [(4, 4), (5, 5), (3, 3), (7, 1), (8, 2), (6, 0)]
B_MAIN, B_LEFT, B_RIGHT, B_TOPBOT, B_C82, B_C60 = range(6)
NB = 6

# edge_in sections (element offsets per partition)
LCOL, RCOL, TOPS, BOTS = 0, 516, 1032, 1548
EIN = 2064
EOUT = 516  # edges_out: [0:128] left, [128:256] right, [256:512] top|bottom,
            # [512:514] corner j01 (d8|d2), [514:516] corner j254/255 (d6|d0)

_CACHE = {}


def _chain(nc, psd, wtr, wslice, slot0, taps, rhs_fn):
    n = len(taps)
    for k, (dy, dx) in enumerate(taps):
        nc.tensor.matmul(psd, wtr[wslice, ts(slot0 + k, 64)], rhs_fn(dy, dx),
                         start=(k == 0), stop=(k == n - 1),
                         skip_group_check=True)


def _build():
    nc = bacc.Bacc("TRN2", target_bir_lowering=False, debug=False,
                   num_devices=NCORES)
    ip = nc.dram_tensor("img_prep", [NSTRIP, 128, XCOLS], BF16,
                        kind="ExternalInput").ap()
    ein_d = nc.dram_tensor("edge_in", [128, EIN], BF16,
                           kind="ExternalInput").ap()
    wtm_d = nc.dram_tensor("wtm", [128, NWM * 64], BF16,
                           kind="ExternalInput").ap()
    wte_d = nc.dram_tensor("wte", [128, NWE * 64], BF16,
                           kind="ExternalInput").ap()
    bias_d = nc.dram_tensor("bias", [128, NB], F32, kind="ExternalInput").ap()
    out_d = nc.dram_tensor("out", [NSTRIP, 2, 128, H2 * W // 2], BF16,
                           kind="ExternalOutput").ap()
    edg_d = nc.dram_tensor("edges", [128, EOUT], BF16,
                           kind="ExternalOutput").ap()

    lo, up = slice(0, 64), slice(64, 128)

    with tile.TileContext(nc) as tc:
        with (
            tc.tile_pool(name="const", bufs=1) as constp,
            tc.tile_pool(name="xin", bufs=5) as xp,
            tc.tile_pool(name="outp", bufs=5) as op,
            tc.tile_pool(name="psmain", bufs=4, space="PSUM") as pp,
        ):
            # ---- startup-critical DMAs first: main weights, strip 0 ----
            wtm = constp.tile([128, NWM * 64], BF16)
            nc.sync.dma_start(wtm[:], wtm_d[:])
            Xs = {}
            Xs[0] = xp.tile([128, XCOLS], BF16, name="X0", tag="X")
            # strip 0 in 3 chunks so group 0 can start after ~1/3 lands
            c1, c2 = 6 * WPAD, 12 * WPAD
            nc.sync.dma_start(Xs[0][:, 0:c1], ip[0][:, 0:c1])
            nc.sync.dma_start(Xs[0][:, c1:c2], ip[0][:, c1:c2])
            nc.sync.dma_start(Xs[0][:, c2:], ip[0][:, c2:])
            bias_t = constp.tile([128, NB], F32)
            nc.sync.dma_start(bias_t[:], bias_d[:])
            for s in (1, 2):
                Xs[s] = xp.tile([128, XCOLS], BF16, name=f"X{s}", tag="X")
                nc.sync.dma_start(Xs[s][:], ip[s])
            # edge tensors stream behind the first strips
            ein = constp.tile([128, EIN], BF16)
            nc.sync.dma_start(ein[:], ein_d[:])
            wte = constp.tile([128, NWE * 64], BF16)
            nc.sync.dma_start(wte[:], wte_d[:])

            # ---- dense interior conv (d=4) ----
            blo = bias_t[:, B_MAIN:B_MAIN + 1]
            for s in range(NSTRIP):
                if s + 3 < NSTRIP:
                    Xs[s + 3] = xp.tile([128, XCOLS], BF16,
                                        name=f"X{s + 3}", tag="X")
                    nc.sync.dma_start(Xs[s + 3][:], ip[s + 3])
                X = Xs.pop(s)
                Xv = X[:].rearrange("p (t m) -> p t m", m=WPAD)
                olo = op.tile([128, H2 * W // 2], BF16, tag="olo")
                oup = op.tile([128, H2 * W // 2], BF16, tag="oup")
                last = s == NSTRIP - 1

                for g in range(NG):
                    ps1 = pp.tile([128, 512], F32, tag="ps1")
                    ps2 = pp.tile([128, 512], F32, tag="ps2")
                    for k, (dy, dx) in enumerate(TAPS9):
                        st, sp = (k == 0), (k == 8)
                        for (ph, po, i) in ((lo, slice(0, 64), 4 * g),
                                            (up, slice(0, 64), 4 * g),
                                            (lo, slice(64, 128), 4 * g + 2),
                                            (up, slice(64, 128), 4 * g + 2)):
                            psd = (ps1 if ph == lo else ps2)
                            rhs = Xv[ph, i + 1 + dy: i + 3 + dy,
                                     dx + 1: dx + 257]
                            nc.tensor.matmul(
                                psd[po, :],
                                wtm[ph, ts(k, 64)], rhs,
                                start=st, stop=sp, skip_group_check=True)
                    # evacuate: bias add psum -> sbuf; alternate engines
                    if g % 2 == 0:
                        nc.scalar.activation(olo[:, ts(g, 512)], ps1[:],
                                             AF.Identity, bias=blo)
                        nc.vector.tensor_scalar_add(oup[:, ts(g, 512)],
                                                    ps2[:], blo)
                    else:
                        nc.vector.tensor_scalar_add(olo[:, ts(g, 512)],
                                                    ps1[:], blo)
                        nc.scalar.activation(oup[:, ts(g, 512)], ps2[:],
                                             AF.Identity, bias=blo)
                    if last:
                        # fine-grained drain so the tail overlaps compute
                        nc.sync.dma_start(out_d[s, 0][:, ts(g, 512)],
                                          olo[:, ts(g, 512)])
                        nc.sync.dma_start(out_d[s, 1][:, ts(g, 512)],
                                          oup[:, ts(g, 512)])
                if not last:
                    nc.sync.dma_start(out_d[s, 0], olo[:])
                    nc.sync.dma_start(out_d[s, 1], oup[:])

            # ---- edge computation (once per image, overlaps drain) ----
            Elc = ein[:, LCOL:LCOL + 516].rearrange("p (t k) -> p t k", k=2)
            Erc = ein[:, RCOL:RCOL + 516].rearrange("p (t k) -> p t k", k=2)
            Et = ein[:, TOPS:TOPS + 516].rearrange("p (r m) -> p r m", m=WPAD)
            Eb = ein[:, BOTS:BOTS + 516].rearrange("p (r m) -> p r m", m=WPAD)
            esb = constp.tile([128, EOUT], BF16)

            # left column (d=5): rows 0-127 on (0,0), rows 128-255 on (64,64)
            pside = pp.tile([128, 256], F32, tag="ps1")
            _chain(nc, pside[lo, 0:128], wte, lo, LEFT_S - NWM, LEFT_TAPS,
                   lambda dy, dx: Elc[lo, 1 + dy: 129 + dy, dx])
            _chain(nc, pside[up, 0:128], wte, up, LEFT_S - NWM, LEFT_TAPS,
                   lambda dy, dx: Elc[up, 129 + dy: 257 + dy, dx])
            nc.scalar.activation(esb[:, 0:128], pside[:, 0:128], AF.Identity,
                                 bias=bias_t[:, B_LEFT:B_LEFT + 1])
            # right column (d=3): img col 255+dx is k index dx+1
            pside2 = pp.tile([128, 256], F32, tag="ps2")
            _chain(nc, pside2[lo, 0:128], wte, lo, RIGHT_S - NWM, RIGHT_TAPS,
                   lambda dy, dx: Erc[lo, 1 + dy: 129 + dy, dx + 1])
            _chain(nc, pside2[up, 0:128], wte, up, RIGHT_S - NWM, RIGHT_TAPS,
                   lambda dy, dx: Erc[up, 129 + dy: 257 + dy, dx + 1])
            nc.scalar.activation(esb[:, 128:256], pside2[:, 0:128], AF.Identity,
                                 bias=bias_t[:, B_RIGHT:B_RIGHT + 1])
            # top row (d=7) on (0,0) / bottom row (d=1) on (64,64)
            ptb = pp.tile([128, 256], F32, tag="ps1")
            _chain(nc, ptb[lo, 0:256], wte, lo, TOP_S - NWM, TOP_TAPS,
                   lambda dy, dx: Et[lo, dy, dx + 1: dx + 257])
            _chain(nc, ptb[up, 0:256], wte, up, BOT_S - NWM, BOT_TAPS,
                   lambda dy, dx: Eb[up, 1 + dy, dx + 1: dx + 257])
            nc.scalar.activation(esb[:, 256:512], ptb[:, 0:256], AF.Identity,
                                 bias=bias_t[:, B_TOPBOT:B_TOPBOT + 1])
            # corners: (0,0) d8 / (255,0) d2 at cols 512:514;
            #          (0,255) d6 / (255,255) d0 at cols 514:516
            pcn = pp.tile([128, 256], F32, tag="ps2")
            _chain(nc, pcn[lo, 0:2], wte, lo, C8_S - NWM, C8_TAPS,
                   lambda dy, dx: Et[lo, dy, dx + 1: dx + 3])
            _chain(nc, pcn[up, 0:2], wte, up, C2_S - NWM, C2_TAPS,
                   lambda dy, dx: Eb[up, 1 + dy, dx + 1: dx + 3])
            _chain(nc, pcn[lo, 2:4], wte, lo, C6_S - NWM, C6_TAPS,
                   lambda dy, dx: Et[lo, dy, 255 + dx: 257 + dx])
            _chain(nc, pcn[up, 2:4], wte, up, C0_S - NWM, C0_TAPS,
                   lambda dy, dx: Eb[up, 1 + dy, 255 + dx: 257 + dx])
            nc.scalar.activation(esb[:, 512:514], pcn[:, 0:2], AF.Identity,
                                 bias=bias_t[:, B_C82:B_C82 + 1])
            nc.scalar.activation(esb[:, 514:516], pcn[:, 2:4], AF.Identity,
                                 bias=bias_t[:, B_C60:B_C60 + 1])
            nc.sync.dma_start(edg_d[:], esb[:])

    nc.compile()
    return nc


def _get_nc():
    if "nc" not in _CACHE:
        _CACHE["nc"] = _build()
    return _CACHE["nc"]


def _prep_img(imgc):
    """[64,256,256] f32 -> [NSTRIP,128,XCOLS] padded bf16 strip layout."""
    ip = np.zeros((NSTRIP, 2, 64, SLOTS, WPAD), BF)
    for s in range(NSTRIP):
        for g in range(2):
            base = s * R + g * H2 - 1
            l0 = max(0, -base)
            h0 = min(SLOTS, H - base)
            ip[s, g, :, l0:h0, 1:257] = imgc[:, base + l0: base + h0, :]
    return np.ascontiguousarray(ip.reshape(NSTRIP, 128, XCOLS))


def _prep_edge_in(imgc):
    L = np.zeros((64, 258, 2), BF)
    L[:, 1:257, :] = imgc[:, :, 0:2]
    Rt = np.zeros((64, 258, 2), BF)
    Rt[:, 1:257, :] = imgc[:, :, 254:256]
    T = np.zeros((64, 2, WPAD), BF)
    T[:, :, 1:257] = imgc[:, 0:2, :]
    Bo = np.zeros((64, 2, WPAD), BF)
    Bo[:, :, 1:257] = imgc[:, 254:256, :]
    half = np.concatenate([L.reshape(64, 516), Rt.reshape(64, 516),
                           T.reshape(64, 516), Bo.reshape(64, 516)], axis=1)
    return np.ascontiguousarray(np.concatenate([half, half], axis=0))


def _prep_wt(weights):
    wt = np.zeros((128, NW, 64), BF)
    for (d, taps), base in zip(W_GROUPS, _offs):
        for k, (dy, dx) in enumerate(taps):
            m = weights[d][:, :, dy + 1, dx + 1].T  # [cin, cout]
            wt[0:64, base + k] = m
            wt[64:128, base + k] = m
    wt = wt.reshape(128, NW * 64)
    return (np.ascontiguousarray(wt[:, :NWM * 64]),
            np.ascontiguousarray(wt[:, NWM * 64:]))


def _prep_bias(bias):
    bs = np.zeros((128, NB), np.float32)
    for c, (dl, du) in enumerate(BIAS_PAIRS):
        bs[0:64, c] = bias[dl]
        bs[64:128, c] = bias[du]
    return bs


def _make_in_maps(img, weights, bias):
    img = np.asarray(img, np.float32)
    wtm, wte = _prep_wt(np.asarray(weights, np.float32))
    bs = _prep_bias(np.asarray(bias, np.float32))
    return [{"img_prep": _prep_img(img[c]), "edge_in": _prep_edge_in(img[c]),
             "wtm": wtm, "wte": wte, "bias": bs}
            for c in range(NCORES)]


def _unprep_out(o, e):
    """Assemble [C,H,W] from dense out + edge overlay."""
    e = e.astype(np.float32)
    v = o.astype(np.float32).reshape(NSTRIP, 2, 2, 64, 4, 2, 256)
    out = np.ascontiguousarray(
        v.transpose(3, 0, 1, 4, 2, 5, 6).reshape(C, H, W))
    Lv = np.concatenate([e[0:64, 0:128], e[64:128, 0:128]], axis=1)
    Rv = np.concatenate([e[0:64, 128:256], e[64:128, 128:256]], axis=1)
    out[:, 1:255, 0] = Lv[:, 1:255]
    out[:, 1:255, 255] = Rv[:, 1:255]
    out[:, 0, 1:255] = e[0:64, 257:511]
    out[:, 255, 1:255] = e[64:128, 257:511]
    out[:, 0, 0] = e[0:64, 512]
    out[:, 255, 0] = e[64:128, 512]
    out[:, 0, 255] = e[0:64, 515]
    out[:, 255, 255] = e[64:128, 515]
    return out


def kernel(img, weights, bias):
    nc = _get_nc()
    in_maps = _make_in_maps(img, weights, bias)
    res = run_bass_kernel_spmd(nc, in_maps, list(range(NCORES)))
    return np.stack([_unprep_out(res.results[c]["out"], res.results[c]["edges"])
                     for c in range(NCORES)])


# revision 14
# speedup vs baseline: 1.1143x; 1.0163x over previous
"""bf16 variant: 4-way PE-quadrant packing + once-per-image edge compute.

Main conv (d=4): four concurrent K=64/M=64 bf16 matmuls occupy the four 64x64
quadrants of the PE array (tile positions (0,0), (0,64), (64,0), (64,64)):
lower/upper image half on array rows, even/odd row-pair on array columns.
Each group of 4 row-pairs runs 9 taps x 4 units; evacuation (bias add,
psum->sbuf) alternates between ScalarE and VectorE.

Timeline engineering:
- PE warmup: ~16 dummy taps on a memset scratch tile run during the initial
  DMA window so the TensorE p-state is at max clock when real data lands.
- Startup-critical DMA order: main-tap weights (148KB), then strip 0 as two
  independent tiles (slots 0-9 / 8-17, 2-slot halo duplicated) so groups 0-1
  start as soon as the first half lands. Edge tensors stream behind strips
  1-2.
- Edge frame (convs d in {7,1,5,3}) is computed once per image AFTER the
  dense loop (overlapping the output-DMA drain), with left/right chains
  crossed over psum partition halves so all four PE quadrants run. The four
  corner pixels are fixed up on the host. Host overlays edges onto the dense
  result. Dense + edge outputs are bf16 (error budget allows it).
"""

import ml_dtypes
import numpy as np

import concourse.bacc as bacc
import concourse.mybir as mybir
import concourse.tile as tile
from concourse.bass import ts
from concourse.bass_utils import run_bass_kernel_spmd

B, C, H, W = 8, 64, 256, 256
NCORES = 8
R = 32              # output rows per strip
H2 = R // 2         # rows per partition-half
NSTRIP = H // R
SLOTS = H2 + 2
WPAD = W + 2
XCOLS = SLOTS * WPAD
NG = H2 // 4        # groups of 4 row-pairs per half
F32 = mybir.dt.float32
BF16 = mybir.dt.bfloat16
AF = mybir.ActivationFunctionType
BF = ml_dtypes.bfloat16
NWARM = 16          # PE p-state warmup taps

TAPS9 = [(dy, dx) for dy in (-1, 0, 1) for dx in (-1, 0, 1)]
TOP_TAPS = [(dy, dx) for dy in (0, 1) for dx in (-1, 0, 1)]      # d=7 row 0
BOT_TAPS = [(dy, dx) for dy in (-1, 0) for dx in (-1, 0, 1)]     # d=1 row 255
LEFT_TAPS = [(dy, dx) for dy in (-1, 0, 1) for dx in (0, 1)]     # d=5 col 0
RIGHT_TAPS = [(dy, dx) for dy in (-1, 0, 1) for dx in (-1, 0)]   # d=3 col 255

W_GROUPS = [
    (4, TAPS9), (7, TOP_TAPS), (1, BOT_TAPS), (5, LEFT_TAPS), (3, RIGHT_TAPS),
]
_offs = []
_acc = 0
for _d, _taps in W_GROUPS:
    _offs.append(_acc)
    _acc += len(_taps)
(MAIN_S, TOP_S, BOT_S, LEFT_S, RIGHT_S) = _offs
NW = _acc  # 33
NWM = len(TAPS9)       # main-tap weight columns (first group)
NWE = NW - NWM         # edge-tap weight columns
# bias tile [128, NB]: column -> (value on partitions 0-63, on 64-127)
BIAS_PAIRS = [(4, 4), (5, 5), (3, 3), (7, 1)]
B_MAIN, B_LEFT, B_RIGHT, B_TOPBOT = range(4)
NB = 4

# edge_in sections (element offsets per partition)
LCOL, RCOL, TOPS, BOTS = 0, 516, 1032, 1548
EIN = 2064
EOUT = 512  # edges_out: [0:128] left, [128:256] right (row-half crossed),
            # [256:512] top|bottom

_CACHE = {}


def _build():
    nc = bacc.Bacc("TRN2", target_bir_lowering=False, debug=False,
                   num_devices=NCORES)
    ip = nc.dram_tensor("img_prep", [NSTRIP, 128, XCOLS], BF16,
                        kind="ExternalInput").ap()
    ein_d = nc.dram_tensor("edge_in", [128, EIN], BF16,
                           kind="ExternalInput").ap()
    wtm_d = nc.dram_tensor("wtm", [128, NWM * 64], BF16,
                           kind="ExternalInput").ap()
    wte_d = nc.dram_tensor("wte", [128, NWE * 64], BF16,
                           kind="ExternalInput").ap()
    bias_d = nc.dram_tensor("bias", [128, NB], F32, kind="ExternalInput").ap()
    out_d = nc.dram_tensor("out", [NSTRIP, 2, 128, H2 * W // 2], BF16,
                           kind="ExternalOutput").ap()
    edg_d = nc.dram_tensor("edges", [128, EOUT], BF16,
                           kind="ExternalOutput").ap()
    warm_d = nc.dram_tensor("warm", [128, 4], F32, kind="ExternalOutput").ap()

    lo, up = slice(0, 64), slice(64, 128)
    quads = ((lo, slice(0, 64)), (up, slice(0, 64)),
             (lo, slice(64, 128)), (up, slice(64, 128)))

    with tile.TileContext(nc) as tc:
        with (
            tc.tile_pool(name="const", bufs=1) as constp,
            tc.tile_pool(name="xin", bufs=5) as xp,
            tc.tile_pool(name="outp", bufs=5) as op,
            tc.tile_pool(name="psmain", bufs=4, space="PSUM") as pp,
        ):
            # ---- PE p-state warmup on a memset scratch (no DMA deps) ----
            if NWARM:
                warm = constp.tile([128, 576], BF16)
                nc.gpsimd.memset(warm[:], 0.0)
                wps1 = pp.tile([128, 512], F32, tag="ps1")
                wps2 = pp.tile([128, 512], F32, tag="ps2")
                for k in range(NWARM):
                    st, sp = (k == 0), (k == NWARM - 1)
                    for (ph, po) in quads:
                        psd = (wps1 if ph == lo else wps2)
                        nc.tensor.matmul(psd[po, :], warm[ph, 0:64],
                                         warm[ph, 64:576], start=st, stop=sp,
                                         skip_group_check=True)
                wsb = constp.tile([128, 4], F32)
                nc.scalar.activation(wsb[:, 0:2], wps1[:, 0:2], AF.Identity)
                nc.scalar.activation(wsb[:, 2:4], wps2[:, 0:2], AF.Identity)

            # ---- startup-critical DMAs: main weights, strip 0 halves ----
            wtm = constp.tile([128, NWM * 64], BF16)
            nc.sync.dma_start(wtm[:], wtm_d[:])
            X0a = constp.tile([128, 10 * WPAD], BF16)
            nc.sync.dma_start(X0a[:], ip[0][:, 0:10 * WPAD])
            X0b = constp.tile([128, 10 * WPAD], BF16)
            nc.sync.dma_start(X0b[:], ip[0][:, 8 * WPAD:18 * WPAD])
            bias_t = constp.tile([128, NB], F32)
            nc.sync.dma_start(bias_t[:], bias_d[:])
            Xs = {}
            for s in (1, 2):
                Xs[s] = xp.tile([128, XCOLS], BF16, name=f"X{s}", tag="X")
                nc.sync.dma_start(Xs[s][:], ip[s])
            # edge tensors + warmup readback stream behind the first strips
            ein = constp.tile([128, EIN], BF16)
            nc.sync.dma_start(ein[:], ein_d[:])
            wte = constp.tile([128, NWE * 64], BF16)
            nc.sync.dma_start(wte[:], wte_d[:])
            if NWARM:
                nc.sync.dma_start(warm_d[:], wsb[:])

            # ---- dense interior conv (d=4) ----
            blo = bias_t[:, B_MAIN:B_MAIN + 1]
            X0av = X0a[:].rearrange("p (t m) -> p t m", m=WPAD)
            X0bv = X0b[:].rearrange("p (t m) -> p t m", m=WPAD)
            for s in range(NSTRIP):
                if s + 3 < NSTRIP:
                    Xs[s + 3] = xp.tile([128, XCOLS], BF16,
                                        name=f"X{s + 3}", tag="X")
                    nc.sync.dma_start(Xs[s + 3][:], ip[s + 3])
                if s == 0:
                    gviews = [(X0av, 0), (X0av, 0), (X0bv, -8), (X0bv, -8)]
                else:
                    Xv = Xs.pop(s)[:].rearrange("p (t m) -> p t m", m=WPAD)
                    gviews = [(Xv, 0)] * NG
                olo = op.tile([128, H2 * W // 2], BF16, tag="olo")
                oup = op.tile([128, H2 * W // 2], BF16, tag="oup")
                last = s == NSTRIP - 1

                for g in range(NG):
                    Xv, ioff = gviews[g]
                    ps1 = pp.tile([128, 512], F32, tag="ps1")
                    ps2 = pp.tile([128, 512], F32, tag="ps2")
                    for k, (dy, dx) in enumerate(TAPS9):
                        st, sp = (k == 0), (k == 8)
                        for (ph, po, i) in ((lo, slice(0, 64), 4 * g),
                                            (up, slice(0, 64), 4 * g),
                                            (lo, slice(64, 128), 4 * g + 2),
                                            (up, slice(64, 128), 4 * g + 2)):
                            psd = (ps1 if ph == lo else ps2)
                            ib = i + ioff
                            rhs = Xv[ph, ib + 1 + dy: ib + 3 + dy,
                                     dx + 1: dx + 257]
                            nc.tensor.matmul(
                                psd[po, :],
                                wtm[ph, ts(k, 64)], rhs,
                                start=st, stop=sp, skip_group_check=True)
                    # evacuate: bias add psum -> sbuf; alternate engines
                    if g % 2 == 0:
                        nc.scalar.activation(olo[:, ts(g, 512)], ps1[:],
                                             AF.Identity, bias=blo)
                        nc.vector.tensor_scalar_add(oup[:, ts(g, 512)],
                                                    ps2[:], blo)
                    else:
                        nc.vector.tensor_scalar_add(olo[:, ts(g, 512)],
                                                    ps1[:], blo)
                        nc.scalar.activation(oup[:, ts(g, 512)], ps2[:],
                                             AF.Identity, bias=blo)
                    if last:
                        # fine-grained drain so the tail overlaps compute
                        nc.sync.dma_start(out_d[s, 0][:, ts(g, 512)],
                                          olo[:, ts(g, 512)])
                        nc.sync.dma_start(out_d[s, 1][:, ts(g, 512)],
                                          oup[:, ts(g, 512)])
                if not last:
                    nc.sync.dma_start(out_d[s, 0], olo[:])
                    nc.sync.dma_start(out_d[s, 1], oup[:])

            # ---- edge frame (once per image, overlaps drain) ----
            Elc = ein[:, LCOL:LCOL + 516].rearrange("p (t k) -> p t k", k=2)
            Erc = ein[:, RCOL:RCOL + 516].rearrange("p (t k) -> p t k", k=2)
            Et = ein[:, TOPS:TOPS + 516].rearrange("p (r m) -> p r m", m=WPAD)
            Eb = ein[:, BOTS:BOTS + 516].rearrange("p (r m) -> p r m", m=WPAD)
            esb = constp.tile([128, EOUT], BF16)

            # left (d=5) on (lo,lo)+(up,up); right (d=3) crossed onto
            # (lo,up)+(up,lo) so all four quadrants stream concurrently.
            # Separate psum tiles per conv: chains sharing psum partitions
            # must not share a bank (pending-zero is per partition x bank).
            psL = pp.tile([128, 128], F32, tag="ps1")
            psR = pp.tile([128, 128], F32, tag="ps2")
            nE = len(LEFT_TAPS)
            for k in range(nE):
                dyl, dxl = LEFT_TAPS[k]
                dyr, dxr = RIGHT_TAPS[k]
                st, sp = (k == 0), (k == nE - 1)
                nc.tensor.matmul(psL[lo, 0:128], wte[lo, ts(LEFT_S - NWM + k, 64)],
                                 Elc[lo, 1 + dyl: 129 + dyl, dxl],
                                 start=st, stop=sp, skip_group_check=True)
                nc.tensor.matmul(psL[up, 0:128], wte[up, ts(LEFT_S - NWM + k, 64)],
                                 Elc[up, 129 + dyl: 257 + dyl, dxl],
                                 start=st, stop=sp, skip_group_check=True)
                nc.tensor.matmul(psR[up, 0:128], wte[lo, ts(RIGHT_S - NWM + k, 64)],
                                 Erc[lo, 1 + dyr: 129 + dyr, dxr + 1],
                                 start=st, stop=sp, skip_group_check=True)
                nc.tensor.matmul(psR[lo, 0:128], wte[up, ts(RIGHT_S - NWM + k, 64)],
                                 Erc[up, 129 + dyr: 257 + dyr, dxr + 1],
                                 start=st, stop=sp, skip_group_check=True)
            nc.scalar.activation(esb[:, 0:128], psL[:, 0:128], AF.Identity,
                                 bias=bias_t[:, B_LEFT:B_LEFT + 1])
            nc.scalar.activation(esb[:, 128:256], psR[:, 0:128], AF.Identity,
                                 bias=bias_t[:, B_RIGHT:B_RIGHT + 1])
            # top row (d=7) on (lo,lo) / bottom row (d=1) on (up,up)
            ptb = pp.tile([128, 256], F32, tag="ps2")
            nT = len(TOP_TAPS)
            for k in range(nT):
                dyt, dxt = TOP_TAPS[k]
                dyb, dxb = BOT_TAPS[k]
                st, sp = (k == 0), (k == nT - 1)
                nc.tensor.matmul(ptb[lo, 0:256], wte[lo, ts(TOP_S - NWM + k, 64)],
                                 Et[lo, dyt, dxt + 1: dxt + 257],
                                 start=st, stop=sp, skip_group_check=True)
                nc.tensor.matmul(ptb[up, 0:256], wte[up, ts(BOT_S - NWM + k, 64)],
                                 Eb[up, 1 + dyb, dxb + 1: dxb + 257],
                                 start=st, stop=sp, skip_group_check=True)
            nc.scalar.activation(esb[:, 256:512], ptb[:, 0:256], AF.Identity,
                                 bias=bias_t[:, B_TOPBOT:B_TOPBOT + 1])
            nc.sync.dma_start(edg_d[:], esb[:])

    nc.compile()
    return nc


def _get_nc():
    if "nc" not in _CACHE:
        _CACHE["nc"] = _build()
    return _CACHE["nc"]


def _prep_img(imgc):
    """[64,256,256] f32 -> [NSTRIP,128,XCOLS] padded bf16 strip layout."""
    ip = np.zeros((NSTRIP, 2, 64, SLOTS, WPAD), BF)
    for s in range(NSTRIP):
        for g in range(2):
            base = s * R + g * H2 - 1
            l0 = max(0, -base)
            h0 = min(SLOTS, H - base)
            ip[s, g, :, l0:h0, 1:257] = imgc[:, base + l0: base + h0, :]
    return np.ascontiguousarray(ip.reshape(NSTRIP, 128, XCOLS))


def _prep_edge_in(imgc):
    L = np.zeros((64, 258, 2), BF)
    L[:, 1:257, :] = imgc[:, :, 0:2]
    Rt = np.zeros((64, 258, 2), BF)
    Rt[:, 1:257, :] = imgc[:, :, 254:256]
    T = np.zeros((64, 2, WPAD), BF)
    T[:, :, 1:257] = imgc[:, 0:2, :]
    Bo = np.zeros((64, 2, WPAD), BF)
    Bo[:, :, 1:257] = imgc[:, 254:256, :]
    half = np.concatenate([L.reshape(64, 516), Rt.reshape(64, 516),
                           T.reshape(64, 516), Bo.reshape(64, 516)], axis=1)
    return np.ascontiguousarray(np.concatenate([half, half], axis=0))


def _prep_wt(weights):
    wt = np.zeros((128, NW, 64), BF)
    for (d, taps), base in zip(W_GROUPS, _offs):
        for k, (dy, dx) in enumerate(taps):
            m = weights[d][:, :, dy + 1, dx + 1].T  # [cin, cout]
            wt[0:64, base + k] = m
            wt[64:128, base + k] = m
    wt = wt.reshape(128, NW * 64)
    return (np.ascontiguousarray(wt[:, :NWM * 64]),
            np.ascontiguousarray(wt[:, NWM * 64:]))


def _prep_bias(bias):
    bs = np.zeros((128, NB), np.float32)
    for c, (dl, du) in enumerate(BIAS_PAIRS):
        bs[0:64, c] = bias[dl]
        bs[64:128, c] = bias[du]
    return bs


def _make_in_maps(img, weights, bias):
    img = np.asarray(img, np.float32)
    wtm, wte = _prep_wt(np.asarray(weights, np.float32))
    bs = _prep_bias(np.asarray(bias, np.float32))
    return [{"img_prep": _prep_img(img[c]), "edge_in": _prep_edge_in(img[c]),
             "wtm": wtm, "wte": wte, "bias": bs}
            for c in range(NCORES)]


def _unprep_out(o, e):
    """Assemble [C,H,W] from dense out + edge overlay (corners excluded)."""
    e = e.astype(np.float32)
    v = o.astype(np.float32).reshape(NSTRIP, 2, 2, 64, 4, 2, 256)
    out = np.ascontiguousarray(
        v.transpose(3, 0, 1, 4, 2, 5, 6).reshape(C, H, W))
    Lv = np.concatenate([e[0:64, 0:128], e[64:128, 0:128]], axis=1)
    # right chain is crossed over psum partition halves
    Rv = np.concatenate([e[64:128, 128:256], e[0:64, 128:256]], axis=1)
    out[:, 1:255, 0] = Lv[:, 1:255]
    out[:, 1:255, 255] = Rv[:, 1:255]
    out[:, 0, 1:255] = e[0:64, 257:511]
    out[:, 255, 1:255] = e[64:128, 257:511]
    return out


def _fix_corners(out, imgc, weights, bias):
    """The 4 corner pixels (convs d in {8,6,2,0}) computed host-side."""
    out[:, 0, 0] = np.einsum('oikl,ikl->o', weights[8][:, :, 1:3, 1:3],
                             imgc[:, 0:2, 0:2]) + bias[8]
    out[:, 0, 255] = np.einsum('oikl,ikl->o', weights[6][:, :, 1:3, 0:2],
                               imgc[:, 0:2, 254:256]) + bias[6]
    out[:, 255, 0] = np.einsum('oikl,ikl->o', weights[2][:, :, 0:2, 1:3],
                               imgc[:, 254:256, 0:2]) + bias[2]
    out[:, 255, 255] = np.einsum('oikl,ikl->o', weights[0][:, :, 0:2, 0:2],
                                 imgc[:, 254:256, 254:256]) + bias[0]


def _assemble(res, img, weights, bias):
    img = np.asarray(img, np.float32)
    weights = np.asarray(weights, np.float32)
    bias = np.asarray(bias, np.float32)
    outs = []
    for c in range(NCORES):
        out = _unprep_out(res.results[c]["out"], res.results[c]["edges"])
        _fix_corners(out, img[c], weights, bias)
        outs.append(out)
    return np.stack(outs)


def kernel(img, weights, bias):
    nc = _get_nc()
    in_maps = _make_in_maps(img, weights, bias)
    res = run_bass_kernel_spmd(nc, in_maps, list(range(NCORES)))
    return _assemble(res, img, weights, bias)
